# revision 38
# baseline (speedup 1.0000x reference)
# Trainium2 Bass kernel for BaseGumbelGraphNetwork message passing.
#
# Reference computation (B=4, N=512, D=2, H=64, O=2):
#   e1 = relu(cat(x_i, x_j) @ W_n2e.T + b_n2e)        [B,N,N,H]
#   e2 = relu(e1 @ W_e2e.T + b_e2e)                   [B,N,N,H]
#   s  = sum_j adj[i,j] * e2                          [B,N,H]
#   h  = relu(relu(s@W_e2n.T+b)@W_n2n.T+b)
#   out= relu(cat(x,h)@W_o1.T+b) @ W_o2.T + b         [B,N,O]
#
# Key structure: layer 1 factorizes over the (i,j) grid:
#   e1[b,i,j,:] = relu(A[b,i,:] + C[b,j,:] + b1),  A = x@Wi.T, C = x@Wj.T
# so the [B,N,N,2D] concat tensor is never materialized.
#
# Device layout (per core, i-dim sharded 8 ways -> 64 rows/core):
#   * a unit is (batch b, i-pair q): partitions = h stacked for the two i's
#     (2x64), free dim = j (512). Two consecutive q's share one [128,1024]
#     tile so ACT/DVE instruction overheads amortize. Loop: b outer, q inner.
#   * per (b, q-pair) iteration:
#       2x DVE tensor_scalar  e1 = relu(C.T + (A_i + b1))   fp16
#       2x PE matmul          e2pre = blockdiag(W_e2e.T).T @ e1  (fp16, N=512)
#       1x ACT                e2 = relu(e2pre + b2)  PSUM -> SBUF bf16 [128,1024]
#       1x DVE tensor_tensor  scr = e2 * adj_bcast             [128,1024] bf16
#          (offloading mask columns to the Pool engine was tried and is a
#          net loss: Q7 tensor_tensor runs at ~2.8ns/col and its latency
#          lands on the reduce matmuls' critical path)
#       2x PE matmul (fused reduce): h1pre[:, b, q] = sum_j W_e2n_bd.T @ scr
#          via a PSUM output AP repeating 4 columns 128 times -- PSUM's
#          per-element has_written accumulate sums all 512 j-columns in
#          hardware (4-column spacing clears the same-address RMW hazard;
#          verified bit-stable vs the conservative 8, and halves the
#          final-stage reduce width).
#   * emission is software-pipelined (TT three iterations behind its ACT,
#     reduce matmuls four behind) since each engine runs its stream in
#     order: at ~100% DVE occupancy a shallow lag lets ACT jitter block
#     the in-order DVE queue and starve the PE.
#   * schedule notes (each worth multiple us on a ~90us kernel):
#       - the 5 weight/input DMAs are emitted BEFORE the 32 adj DMAs: the
#         ~16 DMA completion semaphores are allocated round-robin globally,
#         so emitting adj first chains the weight loads behind 4MB of
#         broadcast traffic.
#       - adj rows are pre-broadcast on the HOST into a [128, T*1024] DRAM
#         tensor; tiles stream in as 32 half-tile DMAs on the otherwise-idle
#         Sync and Pool queues. (Per-row partition_broadcast DMAs on the
#         Scalar queue stalled the ACT sequencer and batch 0 ran 2.6x slow.)
#       - the CTS matmul inputs are fp16 (fp32 matmuls are 4 cycles/row and
#         cost 2.3us at boot-time PE clock).
#       - per-batch setup (CTS = stacked C.T, ABIAS = A + b1) is emitted
#         lazily: batch b+1's setup rides inside batch b's iteration t=1.
#       - a 1-column dummy Relu on ACT absorbs the 1.3us activation-table
#         load; CTS PSUM->SBUF copies run on DVE, keeping ACT's queue clean.
#       - out is written as a contiguous [4, Q] block per batch (4 DMA
#         descriptors instead of 128 scattered 4-byte ones); the host
#         un-permutes.

import numpy as np

B, N, D, H, O = 4, 512, 2, 64, 2
NCORES = 8
IB = N // NCORES  # i rows per core = 64
Q = IB // 2       # i pairs per core = 32
T = Q // 2        # q-pair iterations per batch = 16

_STATE = {}

# wpack column layout (fp32, 128 partitions)
_WP = {}
_o = 0
for _name, _w in [("b1s", 1), ("b2s", 1), ("be2ns", 1), ("bn2ns", 1),
                  ("bo1s", 1), ("bo2s", 1), ("wo2bd", 4),
                  ("wn2nbd", 128), ("wo1hbd", 128)]:
    _WP[_name] = (_o, _o + _w)
    _o += _w
WPACK_COLS = _o
WPACK_SPLIT = _WP["wn2nbd"][0]  # biases chunk loads first (5KB vs 200KB)

# xf16 column layout (fp16, 2 partitions): wjt2, wit, then per-b blocks
XF_FIX = 128 + 64
XFB = N + 2 * Q   # x.T, xtie, xtio widths per b
XF_COLS = XF_FIX + B * XFB
XF_SPLIT = XF_FIX + XFB  # first chunk: fixed + batch 0

# xpk column layout (fp32, 4 partitions; per-b block after the fixed part)
XB = Q            # xpair width per b
XPK_FIX = 128     # wo1xbd
XPK_COLS = XPK_FIX + B * XB
XPK_SPLIT = XPK_FIX + XB  # first chunk: fixed part + batch 0


def _build_nc():
    import concourse.mybir as mybir
    from concourse import bacc
    from concourse.tile import TileContext

    F32 = mybir.dt.float32
    FP16 = mybir.dt.float16   # e1 / W2-matmul path (better weight precision)
    BFL = mybir.dt.bfloat16   # e2 / mask / reduce path (full-rate ACT writes)
    AL = mybir.AluOpType
    AF = mybir.ActivationFunctionType

    nc = bacc.Bacc("TRN2", target_bir_lowering=False, debug=False,
                   num_devices=NCORES)

    def din(name, shape, dt=F32):
        return nc.dram_tensor(name, list(shape), dt, kind="ExternalInput").ap()

    wpack = din("wpack", (128, WPACK_COLS))
    xf16 = din("xf16", (2, XF_COLS), FP16)
    xpk = din("xpk", (4, XPK_COLS))
    adjbc = din("adjbc", (128, T * 1024), BFL)  # host-pre-broadcast adj tiles
    w2bd = din("w2bd", (128, 128), FP16)  # blockdiag(W_e2e.T, W_e2e.T)
    we2nbd = din("we2nbd", (128, 128), BFL)  # blockdiag(W_e2n.T, W_e2n.T)

    # out[b, 2e+o, q] on device; host un-permutes to [B, IB, O]
    out_d = nc.dram_tensor("out", [B, 4, Q], F32, kind="ExternalOutput").ap()

    with TileContext(nc, pool_alloc_mode="queue") as tc:
        with (tc.tile_pool(name="wpool", bufs=1) as wp,
              tc.tile_pool(name="ctsp", bufs=B) as ctsp,
              tc.tile_pool(name="abp", bufs=B) as abp,
              tc.tile_pool(name="adjp", bufs=T) as adjp,
              tc.tile_pool(name="e1p", bufs=12) as e1p,
              tc.tile_pool(name="e2p", bufs=10) as e2p,
              tc.tile_pool(name="scrp", bufs=12) as scrp,
              tc.tile_pool(name="finp", bufs=2) as finp,
              tc.tile_pool(name="psp", bufs=3, space="PSUM") as psp,
              tc.tile_pool(name="smp", bufs=1, space="PSUM") as smp,
              tc.tile_pool(name="hps", bufs=1, space="PSUM") as hps):

            # ---- packed weight/input loads first, all on the Scalar ring
            # (DMA completion sems are a shared pool of ~16 allocated in
            # emission order; adj-first would chain these behind 4MB) ----
            xf16t = wp.tile([2, XF_COLS], FP16, tag="xf16")
            nc.scalar.dma_start(out=xf16t[:, 0:XF_SPLIT],
                                in_=xf16[:, 0:XF_SPLIT])
            # the bias chunk rides the Pool ring: every extra Scalar-ring
            # dispatch would sit ahead of the first e2 activation in ACT's
            # in-order queue (DMA rings exist only on SP/ACT/Pool)
            wpk = wp.tile([128, WPACK_COLS], F32, tag="wpk")
            nc.gpsimd.dma_start(out=wpk[:, 0:WPACK_SPLIT],
                                in_=wpack[:, 0:WPACK_SPLIT])
            # matmul weights ride at the head of the Sync / Pool rings so
            # they don't queue behind the 4MB of adj broadcast traffic
            w2bd_s = wp.tile([128, 128], FP16, tag="w2bd")
            nc.sync.dma_start(out=w2bd_s[:], in_=w2bd[:])
            we2nbd_s = wp.tile([128, 128], BFL, tag="we2nbd")
            nc.gpsimd.dma_start(out=we2nbd_s[:], in_=we2nbd[:])
            # warm the ACT activation table (Relu) before the loop needs it
            warm = wp.tile([128, 1], F32, tag="warm")
            nc.scalar.activation(warm[:], wpk[:, 0:1], AF.Relu)

            # ---- adj tiles: half-tile DMAs on the Sync + Pool rings.
            # Only the first few load up front -- a 4MB burst starves the
            # small weight transfers (completion lags grow to ~4us); the
            # rest stream one tile per batch-0 iteration ----
            ADJ = [None] * T

            def load_adj(t):
                adjt = adjp.tile([128, 1024], BFL, tag="adj", name=f"adj{t}")
                nc.sync.dma_start(out=adjt[:, 0:512],
                                  in_=adjbc[:, 1024 * t:1024 * t + 512])
                nc.gpsimd.dma_start(out=adjt[:, 512:1024],
                                    in_=adjbc[:, 1024 * t + 512:1024 * (t + 1)])
                ADJ[t] = adjt

            ADJ_HEAD = 2
            for t in range(ADJ_HEAD):
                load_adj(t)

            # rest of the packed inputs (batch>=1 setup / final-MLP mats),
            # also off the Scalar ring for the same reason
            nc.gpsimd.dma_start(out=xf16t[:, XF_SPLIT:XF_COLS],
                                in_=xf16[:, XF_SPLIT:XF_COLS])
            xpkt = wp.tile([4, XPK_COLS], F32, tag="xpk")
            nc.gpsimd.dma_start(out=xpkt[:, 0:XPK_SPLIT],
                                in_=xpk[:, 0:XPK_SPLIT])
            nc.gpsimd.dma_start(out=xpkt[:, XPK_SPLIT:XPK_COLS],
                                in_=xpk[:, XPK_SPLIT:XPK_COLS])
            nc.gpsimd.dma_start(out=wpk[:, WPACK_SPLIT:WPACK_COLS],
                                in_=wpack[:, WPACK_SPLIT:WPACK_COLS])

            def wslice(name):
                a, bb = _WP[name]
                return wpk[:, a:bb]
            b1s_s, b2s_s = wslice("b1s"), wslice("b2s")
            be2ns_s, bn2ns_s = wslice("be2ns"), wslice("bn2ns")
            bo1s_s = wslice("bo1s")
            wn2nbd_s, wo1hbd_s = wslice("wn2nbd"), wslice("wo1hbd")
            wo2bd_s = wslice("wo2bd")
            bo2s_s = wpk[0:4, _WP["bo2s"][0]:_WP["bo2s"][1]]
            wjt2_s = xf16t[0:2, 0:128]
            wit_s = xf16t[0:2, 128:192]
            wo1xbd_s = xpkt[0:4, 0:XPK_FIX]

            def xf(b, off, w):
                a = XF_FIX + b * XFB + off
                return xf16t[0:2, a:a + w]

            def xpair(b):
                a = XPK_FIX + b * XB
                return xpkt[0:4, a:a + Q]

            # fused-reduce accumulators: h1pre 8-sub-columns per (b, q);
            # two PSUM banks, one per batch-pair
            h1all = hps.tile([128, 2 * 2 * Q * 4], F32, tag="h1ps",
                             name="h1all")
            h1v = [h1all[:, 256 * i:256 * (i + 1)]
                   .rearrange("p (b q e) -> p b q e", b=2, e=4)
                   for i in range(2)]

            # ---- per-batch setup: CTS (stacked C.T) and ABIAS (A + b1),
            # in 4 sub-stages so the mid-loop emission for batch b+1 never
            # dumps >1us onto one engine in a single iteration ----
            CTS, AB = [None] * B, [None] * B
            _SST = {}

            def setup_s0(b):
                # batch 0 runs before the loop: the main pool is empty, so
                # use it there and keep the 1-buffer smp free of overlap
                if b == 0:
                    ps = psp.tile([128, 1024], F32, tag="ps")
                else:
                    ps = smp.tile([128, 512], F32, tag="sps", name=f"sps{b}")
                nc.tensor.matmul(ps[:, 0:512], lhsT=wjt2_s,
                                 rhs=xf(b, 0, N), start=True, stop=True)
                _SST[b, "cts"] = ps

            def _cast_piece(b, k):
                # on DVE in quarters so the saturated pacer never absorbs
                # more than ~260ns in one iteration (ACT's in-order queue
                # delays e2relu directly; Pool can't read PSUM)
                if k == 0:
                    _SST[b, "ctssb"] = ctsp.tile([128, N], FP16, tag="cts",
                                                 name=f"cts{b}")
                cts = _SST[b, "ctssb"]
                nc.vector.tensor_copy(out=cts[:, 128 * k:128 * (k + 1)],
                                      in_=_SST[b, "cts"][:, 128 * k:128 * (k + 1)])
                if k == 3:
                    CTS[b] = cts

            def setup_s1a(b):
                _cast_piece(b, 0)

            def setup_s1b(b):
                _cast_piece(b, 1)

            def setup_s1c(b):
                _cast_piece(b, 2)

            def setup_s1d(b):
                _cast_piece(b, 3)

            def setup_s2(b):
                if b == 0:
                    ps2 = psp.tile([128, 1024], F32, tag="ps")
                else:
                    ps2 = smp.tile([128, 512], F32, tag="sps",
                                   name=f"spsa{b}")
                nc.tensor.matmul(ps2[0:64, 0:Q], lhsT=wit_s,
                                 rhs=xf(b, N, Q), start=True, stop=True)
                nc.tensor.matmul(ps2[64:128, 0:Q], lhsT=wit_s,
                                 rhs=xf(b, N + Q, Q), start=True, stop=True)
                _SST[b, "ab"] = ps2

            def setup_s3(b):
                ab = abp.tile([128, Q], F32, tag="ab")
                nc.vector.tensor_scalar_add(out=ab[:],
                                            in0=_SST[b, "ab"][0:128, 0:Q],
                                            scalar1=b1s_s)
                AB[b] = ab

            SETUP = [setup_s0, setup_s1a, setup_s1b, setup_s1c,
                     setup_s1d, setup_s2, setup_s3]
            for f in (setup_s0, setup_s2, setup_s1a, setup_s1b,
                      setup_s1c, setup_s1d, setup_s3):
                f(0)
            # batch 1's setup runs eagerly too: the first iterations are
            # PE-ramp-bound, so DVE has slack there -- a free window that
            # beats perturbing batch 0's saturated steady state (uses the
            # isolated smp pool; batch 0 used the main pool, no overlap)
            for f in SETUP:
                f(1)
            # (PE p-state warmup matmuls between setup and the loop were
            # tried twice -- on late-landing and on early-resident data --
            # and both cost more through the in-order PE queue than the
            # ramp they preserved)

            # ---- main loop: b outer, q-pair inner, software pipelined ----
            def emit_tt(jobs):
                for adjt_, e2m_, scrm_ in jobs:
                    nc.vector.tensor_tensor(
                        out=scrm_[:], in0=e2m_[:], in1=adjt_[:], op=AL.mult)

            def emit_red(jobs):
                done_b = None
                for b, t, scrm_, k in jobs:
                    q = 2 * t + k
                    sl = h1v[b // 2][:, b % 2, q:q + 1, :]       # [128, 1, 4]
                    nc.tensor.matmul(sl.broadcast_to((128, 128, 4)),
                                     lhsT=we2nbd_s[:],
                                     rhs=scrm_[:, 512 * k:512 * (k + 1)],
                                     start=True, stop=True)
                    if t == T - 1 and k == 1:
                        done_b = b
                return done_b

            def make_final_stages(b):
                # the output MLP split into 4 stages, one emitted per main
                # loop iteration so the tiny serial chain never blocks the
                # in-order PE/ACT/DVE streams
                st = {}

                def s0():
                    st["h1pre"] = finp.tile([128, Q], F32, tag="h1pre", name=f"h1pre{b}")
                    nc.vector.tensor_reduce(out=st["h1pre"][:],
                                            in_=h1v[b // 2][:, b % 2],
                                            axis=mybir.AxisListType.X,
                                            op=AL.add)
                    st["h1"] = finp.tile([128, Q], F32, tag="h1", name=f"h1_{b}")
                    nc.scalar.activation(st["h1"][:], st["h1pre"][:],
                                         AF.Relu, bias=be2ns_s)

                def s1():
                    ps2 = psp.tile([128, 1024], F32, tag="ps")
                    nc.tensor.matmul(ps2[:, 0:Q], lhsT=wn2nbd_s,
                                     rhs=st["h1"][:], start=True, stop=True)
                    st["h2"] = finp.tile([128, Q], F32, tag="h2", name=f"h2_{b}")
                    nc.scalar.activation(st["h2"][:], ps2[:, 0:Q], AF.Relu,
                                         bias=bn2ns_s)

                def s2():
                    ps3 = psp.tile([128, 1024], F32, tag="ps")
                    nc.tensor.matmul(ps3[:, 0:Q], lhsT=wo1hbd_s,
                                     rhs=st["h2"][:], start=True, stop=False)
                    nc.tensor.matmul(ps3[:, 0:Q], lhsT=wo1xbd_s,
                                     rhs=xpair(b),
                                     start=False, stop=True)
                    st["h3"] = finp.tile([128, Q], F32, tag="h3", name=f"h3_{b}")
                    nc.scalar.activation(st["h3"][:], ps3[:, 0:Q], AF.Relu,
                                         bias=bo1s_s)

                def s3():
                    ps4 = psp.tile([128, 1024], F32, tag="ps")
                    nc.tensor.matmul(ps4[0:4, 0:Q], lhsT=wo2bd_s,
                                     rhs=st["h3"][:], start=True, stop=True)
                    outs = finp.tile([4, Q], F32, tag="outs")
                    nc.scalar.activation(outs[:], ps4[0:4, 0:Q], AF.Identity,
                                         bias=bo2s_s)
                    nc.sync.dma_start(out=out_d[b], in_=outs[:])

                return [s0, s1, s2, s3]

            tt_prev1, tt_prev2, tt_prev3 = [], [], []
            red_prev1, red_prev2, red_prev3, red_prev4 = [], [], [], []
            final_stages = []
            for b in range(B):
                for t in range(T):
                    if b == 0 and ADJ_HEAD + t < T:
                        load_adj(ADJ_HEAD + t)
                    adjt = ADJ[t]
                    e1m = e1p.tile([128, 1024], FP16, tag="e1")
                    for k in range(2):
                        q = 2 * t + k
                        nc.vector.tensor_scalar(
                            out=e1m[:, 512 * k:512 * (k + 1)], in0=CTS[b][:],
                            scalar1=AB[b][:, q:q + 1], scalar2=0.0,
                            op0=AL.add, op1=AL.max)
                    psm = psp.tile([128, 1024], F32, tag="ps")
                    nc.tensor.matmul(psm[:, 0:512], lhsT=w2bd_s[:],
                                     rhs=e1m[:, 0:512], start=True, stop=True)
                    nc.tensor.matmul(psm[:, 512:1024], lhsT=w2bd_s[:],
                                     rhs=e1m[:, 512:1024], start=True,
                                     stop=True)
                    e2m = e2p.tile([128, 1024], BFL, tag="e2")
                    nc.scalar.activation(e2m[:], psm[:], AF.Relu, bias=b2s_s)
                    scrm = scrp.tile([128, 1024], BFL, tag="scr")

                    # TT lags its ACT by 3 iterations and the reduce by 4:
                    # at ~99.9% DVE occupancy a short lag lets any ACT
                    # jitter block the in-order DVE queue (TT -> next TS ->
                    # PE starves), amplifying every hiccup
                    emit_tt(tt_prev3)
                    fb = emit_red(red_prev4)
                    tt_prev3 = tt_prev2
                    tt_prev2 = tt_prev1
                    tt_prev1 = [(adjt, e2m, scrm)]
                    red_prev4 = red_prev3
                    red_prev3 = red_prev2
                    red_prev2 = red_prev1
                    red_prev1 = [(b, t, scrm, 0), (b, t, scrm, 1)]
                    if fb is not None:
                        final_stages.extend(make_final_stages(fb))
                    if final_stages and t % 2 == 0:
                        final_stages.pop(0)()
                    if t in (1, 3, 5, 7, 9, 11, 13) and 1 <= b < B - 1:
                        SETUP[(t - 1) // 2](b + 1)
            for jobs in (tt_prev3, tt_prev2, tt_prev1):
                emit_tt(jobs)
            for jobs in (red_prev4, red_prev3, red_prev2, red_prev1):
                fb = emit_red(jobs)
                if fb is not None:
                    final_stages.extend(make_final_stages(fb))
            for s in final_stages:
                s()

    nc.compile()
    return nc


def _get_nc():
    if "nc" not in _STATE:
        _STATE["nc"] = _build_nc()
    return _STATE["nc"]


def _prep_maps(inputs):
    import ml_dtypes
    bfl = ml_dtypes.bfloat16
    fp16 = np.float16
    f32 = np.float32

    x = np.ascontiguousarray(np.asarray(inputs["input"], f32))      # [B,N,D]
    adj = np.ascontiguousarray(np.asarray(inputs["adj"], f32))      # [N,N]
    W_n2e = np.asarray(inputs["W_n2e"], f32)   # [H, 2D]
    b_n2e = np.asarray(inputs["b_n2e"], f32)
    W_e2e = np.asarray(inputs["W_e2e"], f32)
    b_e2e = np.asarray(inputs["b_e2e"], f32)
    W_e2n = np.asarray(inputs["W_e2n"], f32)
    b_e2n = np.asarray(inputs["b_e2n"], f32)
    W_n2n = np.asarray(inputs["W_n2n"], f32)
    b_n2n = np.asarray(inputs["b_n2n"], f32)
    W_o1 = np.asarray(inputs["W_o1"], f32)     # [H, D+H]
    b_o1 = np.asarray(inputs["b_o1"], f32)
    W_o2 = np.asarray(inputs["W_o2"], f32)     # [O, H]
    b_o2 = np.asarray(inputs["b_o2"], f32)

    Wi, Wj = W_n2e[:, :D], W_n2e[:, D:]

    def bd(w):  # blockdiag(w, w)
        r, c = w.shape
        z = np.zeros((2 * r, 2 * c), f32)
        z[:r, :c] = w
        z[r:, c:] = w
        return z

    wpack = np.zeros((128, WPACK_COLS), f32)

    def put(name, val, rows=128):
        a, bb = _WP[name]
        wpack[:rows, a:bb] = val
    put("b1s", np.concatenate([b_n2e, b_n2e]).reshape(128, 1))
    put("b2s", np.concatenate([b_e2e, b_e2e]).reshape(128, 1))
    put("be2ns", np.concatenate([b_e2n, b_e2n]).reshape(128, 1))
    put("bn2ns", np.concatenate([b_n2n, b_n2n]).reshape(128, 1))
    put("bo1s", np.concatenate([b_o1, b_o1]).reshape(128, 1))
    put("wn2nbd", bd(W_n2n.T))
    put("wo1hbd", bd(W_o1[:, D:].T))
    put("wo2bd", bd(W_o2.T))
    put("bo2s", np.concatenate([b_o2, b_o2]).reshape(4, 1), rows=4)

    adjb = adj.astype(bfl)

    maps = []
    for c in range(NCORES):
        sl = slice(c * IB, (c + 1) * IB)
        xc = x[:, sl]                                    # [B, IB, D]
        xf = np.zeros((2, XF_COLS), fp16)
        xf[:, 0:128] = np.concatenate([Wj.T, Wj.T], axis=1)
        xf[:, 128:192] = Wi.T
        for b in range(B):
            a = XF_FIX + b * XFB
            xf[:, a:a + N] = x[b].T
            xf[:, a + N:a + N + Q] = xc[b, 0::2].T
            xf[:, a + N + Q:a + N + 2 * Q] = xc[b, 1::2].T
        xpk = np.zeros((4, XPK_COLS), f32)
        xpk[0:4, 0:XPK_FIX] = bd(W_o1[:, :D].T)
        for b in range(B):
            a = XPK_FIX + b * XB
            xpk[0:4, a:a + Q] = \
                xc[b].reshape(Q, 2 * D).T                # rows e*2+d
        # host-side broadcast of adj rows into the tile layout the mask
        # TT consumes: tile t quadrant (a = partition half, cc = col half)
        # holds row 4t+2cc+a replicated over 64 partitions
        av = adjb[sl].reshape(T, 2, 2, N)                # [t, cc, a, j]
        adjbc = np.empty((128, T, 1024), bfl)
        for a_ in range(2):
            for cc in range(2):
                adjbc[64 * a_:64 * (a_ + 1), :, 512 * cc:512 * (cc + 1)] = \
                    av[:, cc, a_][None, :, :]
        m = {
            "wpack": wpack,
            "xf16": xf,
            "xpk": xpk,
            "adjbc": adjbc.reshape(128, T * 1024),
            "w2bd": bd(W_e2e.T).astype(fp16),
            "we2nbd": bd(W_e2n.T).astype(bfl),
        }
        maps.append({k: np.ascontiguousarray(v) for k, v in m.items()})
    return maps


def run(inputs, trace=False, **kw):
    from concourse.bass_utils import run_bass_kernel_spmd
    nc = _get_nc()
    maps = _prep_maps(inputs)
    res = run_bass_kernel_spmd(nc, maps, list(range(NCORES)), trace=trace, **kw)
    # device out is [B, 4=(2e+o), Q] per core; un-permute to [B, IB, O]
    outs = []
    for c in range(NCORES):
        oc = res.results[c]["out"].reshape(B, 2, 2, Q)   # [b, e, o, q]
        outs.append(np.transpose(oc, (0, 3, 1, 2)).reshape(B, IB, O))
    out = np.concatenate(outs, axis=1)
    return np.ascontiguousarray(out, dtype=np.float32), res


def kernel(**inputs):
    out, _ = run(inputs, trace=False)
    return out
